# revision 1
# baseline (speedup 1.0000x reference)
"""Trainium2 Bass kernel for nn_ConstraintAwareBiasing.

Computes bias[b, n, i, j] = temp[n] * (relu(relu(hi[b,i] + hj[b,j]) @ W2 + b2) @ W3 + b3)[n]
with hi = x @ W1[:128] + b1, hj = x @ W1[128:], masked by `mask`.

Strategy (8 NeuronCores):
  - Shard the (b, i) query axis: core = b*4 + chunk, each core owns 128 i-rows
    against all 512 j for one batch element.
  - Host precomputes hi/hj (tiny [512,128] matmuls), folds head_temperatures
    into W3, adds b3*temp and applies the mask on the host.
  - On device, per query row i (4 rows = one "group"):
      h1 = relu(hjT + hi_col)        DVE tensor_scalar (bf16, 2x mode)
      p1 = W2^T @ h1                 PE matmul -> PSUM (pairs of i share a
                                     2-bank PSUM tile so one evacuation
                                     instruction covers 2 rows)
      h2 = relu(p1 + b2)             ACT/DVE PSUM->SBUF pass (split 6:2)
      p2[32c:32c+16] = W3'^T @ h2    PE matmul, col-tiled: 4 i-rows pack into
                                     one PSUM bank via tile_position
      s5: slab[.., g] = p2 + b3'     one evacuation pass per 4 rows into a
                                     4-group SBUF slab (ACT/DVE alternating)
      per 4 groups: 4 DMAs (one per col group) ship the slab, split across
      the sync and scalar HWDGE queues; the dram-side AP is transposed to
      (n, g, j) so a single SBUF partition range covers 16 query rows.
    Stage 2 of group g-1 is emitted interleaved with stage 1 of group g
    (software pipelining) so the in-order engine streams don't block.
"""

import numpy as np
import ml_dtypes

import concourse.bass as bass
import concourse.tile as tile
import concourse.mybir as mybir
from concourse import bacc
from concourse.bass_utils import run_bass_kernel_spmd

BF16 = ml_dtypes.bfloat16

B, S, D = 2, 512, 128          # batch, seq, state dim
H, NH = 128, 16                # hidden, heads
N_CORES = 8
CHUNKS = N_CORES // B          # i-chunks per batch element
I_PER_CORE = S // CHUNKS       # 128
GROUPS = I_PER_CORE // 4       # 4 i-rows per group (one PSUM bank of W3 outputs)
NEG_INF = float("-inf")

_CACHE: dict = {}

# Engine-assignment patterns (tuned against NTFF profiles).
# s1 per i (i % len): "v" = VectorE, "g" = GpSimdE, "a" = ScalarE
S1_PAT = ["v"] * 8
# s3 per pair index (pi % len): "a" = ScalarE, "v" = VectorE
S3_PAT = ["a", "v", "a", "a", "a", "v", "a", "a"]  # 6 ACT : 2 DVE
# s5 per group (g % len)
S5_PAT = ["v", "a", "v", "a"]
# groups per output slab: s5 results accumulate in a wide SBUF slab; one DMA
# per PSUM col-group ships a whole slab (4 legal single-partition-range DMAs
# per slab instead of 4 per group)
SLAB_GROUPS = 2


def _build_bass():
    nc = bacc.Bacc("TRN2")
    dt = mybir.dt
    hj_d = nc.dram_tensor("hj", (H, S), dt.bfloat16, kind="ExternalInput")
    hi_d = nc.dram_tensor("hi", (H, I_PER_CORE), dt.float32, kind="ExternalInput")
    w2_d = nc.dram_tensor("w2", (H, H), dt.bfloat16, kind="ExternalInput")
    w3_d = nc.dram_tensor("w3", (H, NH), dt.bfloat16, kind="ExternalInput")
    b2_d = nc.dram_tensor("b2", (H, 1), dt.float32, kind="ExternalInput")
    b3_d = nc.dram_tensor("b3", (H, 1), dt.float32, kind="ExternalInput")
    out_d = nc.dram_tensor("out", (I_PER_CORE, NH, S), dt.float32, kind="ExternalOutput")

    relu = mybir.ActivationFunctionType.Relu
    ident = mybir.ActivationFunctionType.Identity
    add, amax = mybir.AluOpType.add, mybir.AluOpType.max

    with tile.TileContext(nc) as tc:
        with tc.tile_pool(name="singles", bufs=1) as singles, \
             tc.tile_pool(name="h1p", bufs=10) as h1p, \
             tc.tile_pool(name="h2p", bufs=8) as h2p, \
             tc.tile_pool(name="otp", bufs=3) as otp, \
             tc.tile_pool(name="ps1", bufs=3, space="PSUM") as ps1, \
             tc.tile_pool(name="ps2", bufs=2, space="PSUM") as ps2:
            hj = singles.tile([H, S], dt.bfloat16)
            hi = singles.tile([H, I_PER_CORE], dt.float32)
            w2 = singles.tile([H, H], dt.bfloat16)
            w3 = singles.tile([H, NH], dt.bfloat16)
            b2 = singles.tile([H, 1], dt.float32)
            b3 = singles.tile([H, 1], dt.float32)
            # dummy relu first: pulls the ~2.7us ACT table load into the
            # input-DMA wait window instead of serializing at the first s3
            warm = singles.tile([128, 1], dt.float32)
            nc.vector.memset(warm[:], 0.0)
            nc.scalar.activation(out=warm[:], in_=warm[:], func=relu)
            nc.sync.dma_start(out=hj[:], in_=hj_d[:])
            nc.scalar.dma_start(out=hi[:], in_=hi_d[:])
            for t, d in [(w2, w2_d), (w3, w3_d), (b2, b2_d), (b3, b3_d)]:
                nc.sync.dma_start(out=t[:], in_=d[:])

            # 1-group software pipeline: stage2 (W3 matmuls, s5, DMA) of
            # group g-1 is emitted interleaved with stage1 (s1, W2, s3) of
            # group g so in-order engine streams never head-of-line block.
            pend = None   # (g, h2_pair_tiles) awaiting stage2

            def stage2_w3(g, h2g):
                p2 = ps2.tile([128, S], dt.float32, name="p2", tag="p2")
                for c in range(4):
                    nc.tensor.matmul(
                        p2[32 * c:32 * c + NH, :], lhsT=w3[:],
                        rhs=h2g[c // 2][:, (c % 2) * S:(c % 2 + 1) * S],
                        start=True, stop=True, tile_position=(0, 32 * c))
                return p2

            slab_state = {"tile": None}

            def stage2_out(g, p2):
                gs = g % SLAB_GROUPS
                if gs == 0:
                    slab_state["tile"] = otp.tile([128, SLAB_GROUPS * S],
                                                  dt.float32, name="ot", tag="ot")
                ot = slab_state["tile"]
                sl = ot[:, gs * S:(gs + 1) * S]
                if S5_PAT[g % len(S5_PAT)] == "v":
                    nc.vector.tensor_scalar_add(sl, p2[:], b3[:, 0:1])
                else:
                    nc.scalar.activation(out=sl, in_=p2[:], func=ident,
                                         bias=b3[:], scale=1.0)
                if gs == SLAB_GROUPS - 1:
                    # ship the slab: one DMA per PSUM col group c. src is a
                    # contiguous [16, SLAB_GROUPS*S] partition range; dst
                    # iterates (n, g', j) to match: out_d[i0+c :: 4] is
                    # (g', n, j), so transpose the dram-side AP.
                    i0 = 4 * (g - gs)
                    for c in range(4):
                        dst = out_d[i0 + c:i0 + 4 * SLAB_GROUPS:4]
                        dst = dst.rearrange("g n j -> n g j")
                        eng = nc.sync if c < 2 else nc.scalar
                        eng.dma_start(out=dst, in_=ot[32 * c:32 * c + NH, :])

            for g in range(GROUPS + 1):
                if pend is not None:
                    p2 = stage2_w3(*pend)   # PE: inputs ready since last iter

                if g < GROUPS:
                    q = [ps1.tile([H, 2 * S], dt.float32, name=f"q{_p}", tag="q") for _p in range(2)]
                    h2 = [h2p.tile([H, 2 * S], dt.bfloat16, name=f"h2_{_p}", tag="h2") for _p in range(2)]
                    for p in range(2):
                        for c in (2 * p, 2 * p + 1):
                            i = 4 * g + c
                            h1 = h1p.tile([H, S], dt.bfloat16)
                            s1_eng = {"v": nc.vector, "g": nc.gpsimd,
                                      "a": nc.scalar}[S1_PAT[i % len(S1_PAT)]]
                            if s1_eng is nc.scalar:
                                nc.scalar.activation(out=h1[:], in_=hj[:], func=relu,
                                                     bias=hi[:, i:i + 1], scale=1.0)
                            else:
                                s1_eng.tensor_scalar(
                                    out=h1[:], in0=hj[:], scalar1=hi[:, i:i + 1],
                                    scalar2=0.0, op0=add, op1=amax)
                            nc.tensor.matmul(
                                q[p][:, (c % 2) * S:(c % 2 + 1) * S],
                                lhsT=w2[:], rhs=h1[:], start=True, stop=True)
                        pi = 2 * g + p   # global pair index
                        if S3_PAT[pi % len(S3_PAT)] == "a":
                            nc.scalar.activation(out=h2[p][:], in_=q[p][:],
                                                 func=relu, bias=b2[:], scale=1.0)
                        else:
                            nc.vector.tensor_scalar(
                                out=h2[p][:], in0=q[p][:], scalar1=b2[:, 0:1],
                                scalar2=0.0, op0=add, op1=amax)

                if pend is not None:
                    stage2_out(pend[0], p2)
                pend = (g, h2) if g < GROUPS else None
    nc.compile()
    return nc


def _host_prep(inputs):
    x = np.asarray(inputs["state_embeddings"], dtype=np.float32)   # [B, S, D]
    W1 = np.asarray(inputs["W1"], dtype=np.float32)                # [2D, H]
    b1 = np.asarray(inputs["b1"], dtype=np.float32)                # [H]
    W2 = np.asarray(inputs["W2"], dtype=np.float32)                # [H, H]
    b2 = np.asarray(inputs["b2"], dtype=np.float32)                # [H]
    W3 = np.asarray(inputs["W3"], dtype=np.float32)                # [H, NH]
    b3 = np.asarray(inputs["b3"], dtype=np.float32)                # [NH]
    temp = np.asarray(inputs["head_temperatures"], dtype=np.float32)  # [NH]

    hi = x @ W1[:D] + b1                                           # [B, S, H]
    hj = x @ W1[D:]                                                # [B, S, H]
    w3p = (W3 * temp[None, :]).astype(BF16)                        # temp folded in
    b3p = b3 * temp                                                # added on host

    b2col = np.ascontiguousarray(b2.reshape(H, 1))
    b3col = np.zeros((H, 1), dtype=np.float32)                     # unused on device

    in_maps = []
    for core in range(N_CORES):
        b, chunk = divmod(core, CHUNKS)
        i0 = chunk * I_PER_CORE
        in_maps.append({
            "hj": np.ascontiguousarray(hj[b].T).astype(BF16),                  # [H, S]
            "hi": np.ascontiguousarray(hi[b, i0:i0 + I_PER_CORE].T,
                                       dtype=np.float32),                      # [H, I]
            "w2": W2.astype(BF16),
            "w3": w3p,
            "b2": b2col,
            "b3": b3col,
        })
    return in_maps, b3p


def _assemble(results, inputs, b3p):
    mask = np.asarray(inputs["mask"])
    out = np.empty((B, NH, S, S), dtype=np.float32)
    for core in range(N_CORES):
        b, chunk = divmod(core, CHUNKS)
        i0 = chunk * I_PER_CORE
        # core result: [I, NH, S] -> out[b, :, i0:i0+I, :]
        out[b, :, i0:i0 + I_PER_CORE, :] = results[core]["out"].transpose(1, 0, 2)
    if b3p.any():
        out += b3p[None, :, None, None]
    if not mask.all():
        out = np.where(mask[:, None, :, :], out, np.float32(NEG_INF))
    return out


def _get_nc():
    if "nc" not in _CACHE:
        _CACHE["nc"] = _build_bass()
    return _CACHE["nc"]


def run(inputs, trace=False):
    nc = _get_nc()
    in_maps, b3p = _host_prep(inputs)
    res = run_bass_kernel_spmd(nc, in_maps, core_ids=list(range(N_CORES)),
                               trace=trace)
    out = _assemble(res.results, inputs, b3p)
    return out, res


def kernel(**inputs) -> np.ndarray:
    out, _ = run(inputs, trace=False)
    return out

